# revision 8
# baseline (speedup 1.0000x reference)
"""Trainium2 Bass kernel for nn_CellularAutomatonDecoder.

Model (per reference):
  cells = embed[tokens] + pos_embed                        (B, T, D)
  rule_bias MLP from mean(c_states); const_bias = rule_bias @ W1b + b1
  8x CA steps: pre = cells@W1c + roll(cells,+1)@W1l + roll(cells,-1)@W1r + const_bias
               cells = a*cells + (1-a)*tanh(gelu(pre) @ W2 + b2)
  out = LN(cells) @ head_w                                 (B, T, V)

Sharding: pure data-parallel over batch across 8 cores (256 rows each).

Device layout: feature-major SBUF state sigma[d=128 partitions, 8192 tokens],
token order t-major (col j = t*256 + b_local) so the T-axis roll becomes a
+-256-column shift with a single wraparound piece -> all matmuls contiguous.
State is kept scaled: sigma = cells / (1-a), so the leaky blend becomes the
single fused DVE op  sigma' = a*sigma + tanh(...), with (1-a) folded into the
activation scales and pre-scaled weights.
"""

import os
import sys

import numpy as np

for _p in ("/opt/trn_rl_repo", "/root/.axon_site/_ro/trn_rl_repo"):
    if os.path.isdir(_p) and _p not in sys.path:
        sys.path.append(_p)

from contextlib import ExitStack

import concourse.bacc as bacc
import concourse.tile as tile
from concourse import mybir
from concourse.bass_utils import run_bass_kernel_spmd

F32 = mybir.dt.float32
F32R = mybir.dt.float32r
AF = mybir.ActivationFunctionType
ALU = mybir.AluOpType
AX = mybir.AxisListType

B, T, D, V, CDIM = 2048, 32, 128, 256, 128
NEV = 8
EPS = 1e-5
NC = 8
BL = B // NC          # 256 batch rows per core
NTOK = BL * T         # 8192 tokens per core
CH = 1024             # token chunk (columns)
NCH = NTOK // CH      # 8 chunks
NBLK = NTOK // 128    # 64 head blocks

TRACE = False         # test harness may flip this (with prof shim installed)
_CACHE = {}


def _pieces(dst0, n, shift):
    """Contiguous (dst, src, len) pieces of src = (dst + shift) mod NTOK."""
    out = []
    j = 0
    while j < n:
        s = (dst0 + j + shift) % NTOK
        ln = min(n - j, NTOK - s)
        out.append((dst0 + j, s, ln))
        j += ln
    return out


def _build(a, has_lnb):
    ia = 1.0 - a
    nc = bacc.Bacc("TRN2", target_bir_lowering=False, debug=False, num_devices=NC)

    def din(name, shape, dt=F32):
        return nc.dram_tensor(name, list(shape), dt, kind="ExternalInput").ap()

    tok_d = din("tok", (1, NTOK), F32R)
    wc_d = din("wc", (128, 256), F32R)
    wl_d = din("wl", (128, 256), F32R)
    wr_d = din("wr", (128, 256), F32R)
    w1b_d = din("w1b", (128, 256))
    w2_d = din("w2", (128, 256), F32R)
    emb_d = din("emb", (128, 256), F32R)
    hwc_d = din("hwc", (128, 256), F32R)
    wc1_d = din("wc1", (128, 256))
    wc2_d = din("wc2", (128, 256))
    posT_d = din("posT", (128, T))
    cT_d = din("cT", (128, 4))
    bc1_d = din("bc1", (128, 2))
    bc2_d = din("bc2", (128, 1))
    b1_d = din("b1", (128, 2))
    b2_d = din("b2", (128, 1))
    vid_d = din("vid", (128, 2))
    ones_d = din("ones", (128, 128), F32R)
    eye2_d = din("eye2", (2, 2))
    if has_lnb:
        consth_d = din("consth", (1, V), F32R)
    out_d = nc.dram_tensor("out", [NTOK, V], F32, kind="ExternalOutput").ap()
    out_r = out_d.rearrange("(b t) v -> b t v", t=T)

    with tile.TileContext(nc) as tc, ExitStack() as ctx:
        # ---- persistent SBUF ----
        wpool = ctx.enter_context(tc.tile_pool(name="weights", bufs=1))

        def wtile(dram_ap, shape, tag, dt=F32):
            t_ = wpool.tile(list(shape), dt, tag=tag, name=tag)
            nc.sync.dma_start(t_[:], dram_ap)
            return t_

        wc_s = wtile(wc_d, (128, 256), "wc", F32R)
        wl_s = wtile(wl_d, (128, 256), "wl", F32R)
        wr_s = wtile(wr_d, (128, 256), "wr", F32R)
        w1b_s = wtile(w1b_d, (128, 256), "w1b")
        w2_s = wtile(w2_d, (128, 256), "w2", F32R)
        emb_s = wtile(emb_d, (128, 256), "emb", F32R)
        hwc_s = wtile(hwc_d, (128, 256), "hwc", F32R)
        wc1_s = wtile(wc1_d, (128, 256), "wc1")
        wc2_s = wtile(wc2_d, (128, 256), "wc2")
        posT_s = wtile(posT_d, (128, T), "posT")
        cT_s = wtile(cT_d, (128, 4), "cT")
        bc1_s = wtile(bc1_d, (128, 2), "bc1")
        bc2_s = wtile(bc2_d, (128, 1), "bc2")
        b1_s = wtile(b1_d, (128, 2), "b1")
        b2_s = wtile(b2_d, (128, 1), "b2")
        vid_s = wtile(vid_d, (128, 2), "vid")
        eye2_s = wtile(eye2_d, (2, 2), "eye2")
        if has_lnb:
            consth_s = wtile(consth_d, (1, V), "consth", F32R)

        ones_s = wtile(ones_d, (128, 128), "ones", F32R)

        spool = ctx.enter_context(tc.tile_pool(name="state", bufs=1))
        sig = spool.tile([128, NTOK], F32R, tag="sigma")

        mlp_sb = ctx.enter_context(tc.tile_pool(name="mlp_sb", bufs=1))
        cbias_s = mlp_sb.tile([128, 2], F32, tag="cbias")

        # ---- rule-bias MLP + token gather (init PSUM scope) ----
        with tc.tile_pool(name="psum_i", bufs=2, space="PSUM") as ppi, \
             tc.tile_pool(name="init_sb", bufs=4) as sbi, \
             tc.tile_pool(name="tok_sb", bufs=2) as sbtok:
            # rule MLP: all tiny column ops
            cp_s = mlp_sb.tile([128, 1], F32, tag="cp")
            nc.vector.tensor_reduce(cp_s[:], cT_s[:], axis=AX.X, op=ALU.add)
            y1_ps = ppi.tile([128, 2], F32, tag="mlp")
            for h in range(2):
                nc.tensor.matmul(y1_ps[:, h:h + 1], (wc1_s[:, h * 128:(h + 1) * 128]),
                                 (cp_s[:]), start=True, stop=True)
            y1g_s = mlp_sb.tile([128, 2], F32, tag="y1g")
            for h in range(2):
                nc.scalar.activation(y1g_s[:, h:h + 1], y1_ps[:, h:h + 1], AF.Gelu,
                                     bias=bc1_s[:, h:h + 1], scale=0.25)
            rb_ps = ppi.tile([128, 2], F32, tag="mlp")
            nc.tensor.matmul(rb_ps[:, 0:1], (wc2_s[:, 0:128]), (y1g_s[:, 0:1]),
                             start=True, stop=False)
            nc.tensor.matmul(rb_ps[:, 0:1], (wc2_s[:, 128:256]), (y1g_s[:, 1:2]),
                             start=False, stop=True)
            rb_s = mlp_sb.tile([128, 1], F32, tag="rb")
            nc.scalar.activation(rb_s[:], rb_ps[:, 0:1], AF.Identity, bias=bc2_s[:, 0:1])
            cb_ps = ppi.tile([128, 2], F32, tag="mlp")
            for h in range(2):
                nc.tensor.matmul(cb_ps[:, h:h + 1], (w1b_s[:, h * 128:(h + 1) * 128]),
                                 (rb_s[:]), start=True, stop=True)
            for h in range(2):
                nc.scalar.activation(cbias_s[:, h:h + 1], cb_ps[:, h:h + 1], AF.Identity,
                                     bias=b1_s[:, h:h + 1])

            # gather: sigma0 = (embed[tok] + pos) / (1-a), via one-hot matmuls
            for ci in range(NCH):
                c0 = ci * CH
                tok_t = sbtok.tile([1, CH], F32R, tag="tok")
                nc.sync.dma_start(tok_t[:], tok_d[0:1, c0:c0 + CH])
                oh_lo = sbi.tile([128, CH], F32R, tag="oh")
                oh_hi = sbi.tile([128, CH], F32R, tag="oh")
                for k in range(2):
                    jc = slice(k * 512, (k + 1) * 512)
                    tb_ps = ppi.tile([128, 512], F32, tag="tokb")
                    nc.tensor.matmul(tb_ps[:], (ones_s[0:1, 0:128]), (tok_t[0:1, jc]),
                                     start=True, stop=True)
                    nc.vector.tensor_scalar(oh_lo[:, jc], tb_ps[:], vid_s[:, 0:1], None,
                                            ALU.is_equal)
                    nc.vector.tensor_scalar(oh_hi[:, jc], tb_ps[:], vid_s[:, 1:2], None,
                                            ALU.is_equal)
                cells_ps = ppi.tile([128, CH], F32, tag="cells")
                for k in range(2):
                    jc = slice(k * 512, (k + 1) * 512)
                    nc.tensor.matmul(cells_ps[:, jc], (emb_s[:, 0:128]), (oh_lo[:, jc]),
                                     start=True, stop=False)
                    nc.tensor.matmul(cells_ps[:, jc], (emb_s[:, 128:256]), (oh_hi[:, jc]),
                                     start=False, stop=True)
                for kb in range(CH // 256):
                    # col j = t*256 + b_local  ->  t = j // 256
                    tt = (c0 + kb * 256) // 256
                    nc.scalar.activation(sig[:, c0 + kb * 256: c0 + (kb + 1) * 256],
                                         cells_ps[:, kb * 256:(kb + 1) * 256],
                                         AF.Identity, bias=posT_s[:, tt:tt + 1])

        # ---- evolve: 8 CA steps ----
        with tc.tile_pool(name="psum_pre", bufs=3, space="PSUM") as ppre, \
             tc.tile_pool(name="psum_new", bufs=1, space="PSUM") as pnew, \
             tc.tile_pool(name="h_sb", bufs=4) as sbh, \
             tc.tile_pool(name="t_sb", bufs=NCH + 1) as sbt:
            for s in range(NEV):
                t_tiles = []
                for ci in range(NCH):
                    c0 = ci * CH
                    pre = [ppre.tile([128, CH], F32, tag="pre", name=f"pre{h_}") for h_ in range(2)]
                    for h in range(2):
                        hcols = slice(h * 128, (h + 1) * 128)
                        for k in range(2):
                            d0 = c0 + k * 512
                            segs = [(wc_s, [(d0, d0, 512)]),
                                    (wl_s, _pieces(d0, 512, -256)),
                                    (wr_s, _pieces(d0, 512, +256))]
                            flat = [(w, dd, ss, ll) for w, ps in segs for dd, ss, ll in ps]
                            for i, (w, dd, ss, ll) in enumerate(flat):
                                nc.tensor.matmul(
                                    pre[h][:, dd - c0: dd - c0 + ll],
                                    (w[:, hcols]), (sig[:, ss:ss + ll]),
                                    start=(i == 0), stop=(i == len(flat) - 1))
                    h_t = [sbh.tile([128, CH], F32R, tag="h", name=f"ht{h_}") for h_ in range(2)]
                    for h in range(2):
                        nc.scalar.activation(h_t[h][:], pre[h][:], AF.Gelu,
                                             bias=cbias_s[:, h:h + 1], scale=ia)
                    new_ps = pnew.tile([128, CH], F32, tag="new")
                    for k in range(2):
                        jc = slice(k * 512, (k + 1) * 512)
                        nc.tensor.matmul(new_ps[:, jc], (w2_s[:, 0:128]), (h_t[0][:, jc]),
                                         start=True, stop=False)
                        nc.tensor.matmul(new_ps[:, jc], (w2_s[:, 128:256]), (h_t[1][:, jc]),
                                         start=False, stop=True)
                    t_t = sbt.tile([128, CH], F32, tag="t")
                    nc.scalar.activation(t_t[:], new_ps[:], AF.Tanh, bias=b2_s[:, 0:1])
                    t_tiles.append(t_t)
                # blends at end of step (keeps per-chunk pipeline free of
                # sigma-write hazards within the step)
                for ci in range(NCH):
                    c0 = ci * CH
                    nc.vector.scalar_tensor_tensor(
                        sig[:, c0:c0 + CH], sig[:, c0:c0 + CH], a, t_tiles[ci][:],
                        op0=ALU.mult, op1=ALU.add)

        # ---- final: LayerNorm stats + head ----
        with tc.tile_pool(name="psum_sr", bufs=1, space="PSUM") as psr, \
             tc.tile_pool(name="psum_tm", bufs=1, space="PSUM") as ptm, \
             tc.tile_pool(name="psum_A", bufs=3, space="PSUM") as pA, \
             tc.tile_pool(name="sq_sb", bufs=2) as sbsq, \
             tc.tile_pool(name="srow_sb", bufs=2) as sbsr, \
             tc.tile_pool(name="stat_sb", bufs=1) as sbst, \
             tc.tile_pool(name="out_sb", bufs=4) as sbo:
            stats_tm = ptm.tile([128, 2 * NBLK], F32, tag="stm")
            for ci in range(NCH):
                c0 = ci * CH
                sq_t = sbsq.tile([128, CH], F32R, tag="sq")
                nc.scalar.activation(sq_t[:], sig[:, c0:c0 + CH], AF.Square)
                sr_ps = psr.tile([1, 2 * CH], F32, tag="srow")
                for k in range(2):
                    jc = slice(k * 512, (k + 1) * 512)
                    jc2 = slice(CH + k * 512, CH + (k + 1) * 512)
                    nc.tensor.matmul(sr_ps[0:1, jc], (ones_s[:, 0:1]),
                                     (sig[:, c0 + k * 512:c0 + (k + 1) * 512]),
                                     start=True, stop=True)
                    nc.tensor.matmul(sr_ps[0:1, jc2], (ones_s[:, 0:1]), (sq_t[:, jc]),
                                     start=True, stop=True)
                srow_t = sbsr.tile([1, 2 * CH], F32, tag="srow")
                nc.scalar.activation(srow_t[0:1, 0:CH], sr_ps[0:1, 0:CH], AF.Copy)
                nc.vector.tensor_copy(srow_t[0:1, CH:2 * CH], sr_ps[0:1, CH:2 * CH])
                for j in range(CH // 128):
                    b = ci * (CH // 128) + j
                    nc.tensor.transpose(stats_tm[:, 2 * b:2 * b + 1],
                                        srow_t[0:1, j * 128:(j + 1) * 128],
                                        eye2_s[0:1, 0:1])
                    nc.tensor.transpose(stats_tm[:, 2 * b + 1:2 * b + 2],
                                        srow_t[0:1, CH + j * 128:CH + (j + 1) * 128],
                                        eye2_s[0:1, 0:1])
            # per-token inv-std on [128, 64] tiles (token-major)
            st3 = stats_tm[:].rearrange("p (b two) -> p b two", two=2)
            s1ap = st3[:, :, 0]
            s2ap = st3[:, :, 1]
            m2_s = sbst.tile([128, NBLK], F32, tag="m2")
            nc.scalar.activation(m2_s[:], s1ap, AF.Square, scale=ia / 128.0)
            vf_s = sbst.tile([128, NBLK], F32, tag="vf")
            nc.vector.scalar_tensor_tensor(vf_s[:], s2ap, ia * ia / 128.0, m2_s[:],
                                           op0=ALU.mult, op1=ALU.subtract)
            nc.vector.tensor_scalar_add(vf_s[:], vf_s[:], EPS)
            sd_s = sbst.tile([128, NBLK], F32, tag="sd")
            nc.scalar.activation(sd_s[:], vf_s[:], AF.Sqrt)
            y0_s = sbst.tile([128, NBLK], F32, tag="y0")
            nc.vector.reciprocal(y0_s[:], sd_s[:])
            q_s = sbst.tile([128, NBLK], F32, tag="q")
            nc.vector.tensor_mul(q_s[:], y0_s[:], y0_s[:])
            w_s = sbst.tile([128, NBLK], F32, tag="w")
            nc.vector.scalar_tensor_tensor(w_s[:], vf_s[:], -0.5, q_s[:],
                                           op0=ALU.mult, op1=ALU.mult)
            inv_s = sbst.tile([128, NBLK], F32, tag="inv")
            nc.vector.scalar_tensor_tensor(inv_s[:], w_s[:], 1.5, y0_s[:],
                                           op0=ALU.add, op1=ALU.mult)
            # head: out[tok, v] = inv[tok] * (sigma_blk^T @ hwc) (+ ln_b @ head_w)
            for b in range(NBLK):
                A_ps = pA.tile([128, V], F32, tag="A")
                nc.tensor.matmul(A_ps[:], (sig[:, b * 128:(b + 1) * 128]), (hwc_s[:]),
                                 start=True, stop=(not has_lnb))
                if has_lnb:
                    nc.tensor.matmul(A_ps[:], (ones_s[0:1, 0:128]), (consth_s[:]),
                                     start=False, stop=True)
                o_t = sbo.tile([128, V], F32, tag="o")
                nc.vector.tensor_scalar(o_t[:], A_ps[:], inv_s[:, b:b + 1], None, ALU.mult)
                tt = b // 2
                b0 = (b % 2) * 128
                nc.sync.dma_start(out_r[b0:b0 + 128, tt, :], o_t[:])

    nc.compile()
    return nc


def kernel(**inputs):
    g = {k: np.asarray(v, np.float32) if k != "tokens" else np.asarray(v)
         for k, v in inputs.items()}
    alpha = float(g["alpha"])
    a = float(1.0 / (1.0 + np.exp(-np.float64(alpha))))
    ia = 1.0 - a
    ln_b = g["ln_b"]
    has_lnb = bool(np.any(ln_b != 0))
    key = (np.float64(a).tobytes(), has_lnb)
    if key not in _CACHE:
        _CACHE[key] = _build(a, has_lnb)
    nc = _CACHE[key]

    W1, W2 = g["W1"], g["W2"]
    embed, pos = g["embed"], g["pos_embed"]
    head_w, ln_g = g["head_w"], g["ln_g"]
    shared = {
        "wc": np.ascontiguousarray(W1[0:128]),
        "wl": np.ascontiguousarray(W1[128:256]),
        "wr": np.ascontiguousarray(W1[256:384]),
        "w1b": np.ascontiguousarray(W1[384:512]),
        "w2": np.concatenate([W2[0:128], W2[128:256]], axis=1),
        "emb": np.concatenate([embed[0:128], embed[128:256]], axis=1) * np.float32(1.0 / ia),
        "posT": np.ascontiguousarray(pos.T) * np.float32(1.0 / ia),
        "wc1": np.ascontiguousarray(g["Wc1"]),
        "wc2": np.concatenate([g["Wc2"][0:128], g["Wc2"][128:256]], axis=1),
        "cT": np.ascontiguousarray(g["c_states"].T),
        "bc1": np.ascontiguousarray(g["bc1"].reshape(2, 128).T),
        "bc2": g["bc2"].reshape(128, 1),
        "b1": np.ascontiguousarray(g["b1"].reshape(2, 128).T),
        "b2": g["b2"].reshape(128, 1),
        "vid": np.stack([np.arange(128), np.arange(128, 256)], axis=1).astype(np.float32),
        "ones": np.ones((128, 128), np.float32),
        "eye2": np.eye(2, dtype=np.float32),
    }
    ghw = head_w * ln_g[:, None]
    shared["hwc"] = (ghw - ghw.mean(axis=0, keepdims=True)) * np.float32(ia)
    if has_lnb:
        shared["consth"] = (ln_b @ head_w).reshape(1, V)
    shared = {k: np.ascontiguousarray(v, np.float32) for k, v in shared.items()}

    tokens = g["tokens"]
    in_maps = []
    for c in range(NC):
        tk = tokens[c * BL:(c + 1) * BL].astype(np.float32)   # (BL, T)
        m = dict(shared)
        m["tok"] = np.ascontiguousarray(tk.T).reshape(1, NTOK)  # t-major
        in_maps.append(m)

    kw = {}
    if TRACE:
        kw = dict(trace=True)
    res = run_bass_kernel_spmd(nc, in_maps, core_ids=list(range(NC)), **kw)
    if TRACE and res.exec_time_ns is not None:
        print(f"HW exec time: {res.exec_time_ns} ns")
        kernel.last_exec_ns = res.exec_time_ns
        kernel.last_trace = res.instructions_and_trace
    out = np.stack([res.results[c]["out"] for c in range(NC)], axis=0)
    return np.ascontiguousarray(out.reshape(B, T, V))


# revision 10
# speedup vs baseline: 1.0564x; 1.0564x over previous
"""Trainium2 Bass kernel for nn_CellularAutomatonDecoder.

Model (per reference):
  cells = embed[tokens] + pos_embed                        (B, T, D)
  rule_bias MLP from mean(c_states); const_bias = rule_bias @ W1b + b1
  8x CA steps: pre = cells@W1c + roll(cells,+1)@W1l + roll(cells,-1)@W1r + const_bias
               cells = a*cells + (1-a)*tanh(gelu(pre) @ W2 + b2)
  out = LN(cells) @ head_w                                 (B, T, V)

Sharding: pure data-parallel over batch across 8 cores (256 rows each).

Device layout: feature-major SBUF state sigma[d=128 partitions, 8192 tokens],
token order t-major (col j = t*256 + b_local) so the T-axis roll becomes a
+-256-column shift with a single wraparound piece -> all matmuls contiguous
fp32r at N>=256. State is kept scaled: sigma = cells / (1-a), so the leaky
blend is a single fused DVE op  sigma' = a*sigma + tanh(...), with (1-a)
folded into activation scales and pre-scaled weights. The head runs with
sigma blocks as the stationary operand so the output lands token-major in
PSUM and DMAs out contiguously; LayerNorm reduces to a per-token inv-std
scale after folding ln_g and the mean-removal into a column-centered head
weight matrix.
"""

import os
import sys

import numpy as np

for _p in ("/opt/trn_rl_repo", "/root/.axon_site/_ro/trn_rl_repo"):
    if os.path.isdir(_p) and _p not in sys.path:
        sys.path.append(_p)

from contextlib import ExitStack

import concourse.bacc as bacc
import concourse.tile as tile
from concourse import mybir
from concourse.bass_utils import run_bass_kernel_spmd

F32 = mybir.dt.float32
F32R = mybir.dt.float32r
AF = mybir.ActivationFunctionType
ALU = mybir.AluOpType
AX = mybir.AxisListType

B, T, D, V, CDIM = 2048, 32, 128, 256, 128
NEV = 8
EPS = 1e-5
NC = 8
BL = B // NC          # 256 batch rows per core
NTOK = BL * T         # 8192 tokens per core
CH = 1024             # token chunk (columns)
NCH = NTOK // CH      # 8 chunks
NBLK = NTOK // 128    # 64 head blocks

# packed f32r weights: columns in wpack [128, 1792]
_WOFF = {"wc": 0, "wl": 256, "wr": 512, "w2": 768, "emb": 1024, "hwc": 1280,
         "ones": 1536}
WPACK_W = 1536 + 128
# packed f32 consts: columns in fpack [128, 896]
_FOFF = {"w1b": 0, "wc1": 256, "wc2": 512, "posT": 768, "cT": 800, "bc1": 804,
         "bc2": 806, "b1": 807, "b2": 809, "vid": 810, "eye2": 812,
         "consth": 814}
FPACK_W = 814 + 256

TRACE = False         # test harness may flip this (with prof shim installed)
_CACHE = {}


def _pieces(dst0, n, shift):
    """Contiguous (dst, src, len) pieces of src = (dst + shift) mod NTOK."""
    out = []
    j = 0
    while j < n:
        s = (dst0 + j + shift) % NTOK
        ln = min(n - j, NTOK - s)
        out.append((dst0 + j, s, ln))
        j += ln
    return out


def _build(a, has_lnb):
    ia = 1.0 - a
    nc = bacc.Bacc("TRN2", target_bir_lowering=False, debug=False, num_devices=NC)

    tok_d = nc.dram_tensor("tok", [1, NTOK], F32R, kind="ExternalInput").ap()
    wpack_d = nc.dram_tensor("wpack", [128, WPACK_W], F32R, kind="ExternalInput").ap()
    fpack_d = nc.dram_tensor("fpack", [128, FPACK_W], F32, kind="ExternalInput").ap()
    out_d = nc.dram_tensor("out", [NTOK, V], F32, kind="ExternalOutput").ap()
    out_r = out_d.rearrange("(b t) v -> b t v", t=T)

    with tile.TileContext(nc) as tc, ExitStack() as ctx:
        # ---- persistent SBUF ----
        wpool = ctx.enter_context(tc.tile_pool(name="weights", bufs=1))
        wpack = wpool.tile([128, WPACK_W], F32R, tag="wpack")
        nc.sync.dma_start(wpack[:], wpack_d)
        fpack = wpool.tile([128, FPACK_W], F32, tag="fpack")
        nc.sync.dma_start(fpack[:], fpack_d)

        def W(nm, w=256):
            o = _WOFF[nm]
            return wpack[:, o:o + w]

        def F(nm, w):
            o = _FOFF[nm]
            return fpack[:, o:o + w]

        wc_s, wl_s, wr_s, w2_s = W("wc"), W("wl"), W("wr"), W("w2")
        emb_s, hwc_s, ones_s = W("emb"), W("hwc"), W("ones", 128)
        w1b_s, wc1_s, wc2_s = F("w1b", 256), F("wc1", 256), F("wc2", 256)
        posT_s, cT_s = F("posT", 32), F("cT", 4)
        bc1_s, bc2_s = F("bc1", 2), F("bc2", 1)
        b1_s, b2_s, vid_s = F("b1", 2), F("b2", 1), F("vid", 2)
        eye2_s = fpack[0:2, _FOFF["eye2"]:_FOFF["eye2"] + 2]
        consth_s = fpack[0:1, _FOFF["consth"]:_FOFF["consth"] + 2 * 128]  # f32 row
        # consth is consumed as f32r by a matmul; reuse wpack's ones region?
        # simpler: keep const MM reading a bitcast view (dtype f32r ok: DMA'd).
        consth_r = consth_s.bitcast(F32R)

        spool = ctx.enter_context(tc.tile_pool(name="state", bufs=1))
        sig = spool.tile([128, NTOK], F32R, tag="sigma")

        mlp_sb = ctx.enter_context(tc.tile_pool(name="mlp_sb", bufs=1))
        cbias_s = mlp_sb.tile([128, 2], F32, tag="cbias")

        # ---- token gather + rule-bias MLP (init PSUM scope) ----
        with tc.tile_pool(name="psum_i", bufs=2, space="PSUM") as ppi, \
             tc.tile_pool(name="init_sb", bufs=4) as sbi, \
             tc.tile_pool(name="tok_sb", bufs=2) as sbtok:
            # gather: sigma0 = (embed[tok] + pos) / (1-a), via one-hot matmuls
            for ci in range(NCH):
                c0 = ci * CH
                tok_t = sbtok.tile([1, CH], F32R, tag="tok")
                nc.sync.dma_start(tok_t[:], tok_d[0:1, c0:c0 + CH])
                oh_lo = sbi.tile([128, CH], F32R, tag="oh")
                oh_hi = sbi.tile([128, CH], F32R, tag="oh")
                for k in range(2):
                    jc = slice(k * 512, (k + 1) * 512)
                    tb_ps = ppi.tile([128, 512], F32, tag="tokb")
                    nc.tensor.matmul(tb_ps[:], ones_s[0:1, 0:128], tok_t[0:1, jc],
                                     start=True, stop=True)
                    nc.vector.tensor_scalar(oh_lo[:, jc], tb_ps[:], vid_s[:, 0:1], None,
                                            ALU.is_equal)
                    nc.vector.tensor_scalar(oh_hi[:, jc], tb_ps[:], vid_s[:, 1:2], None,
                                            ALU.is_equal)
                cells_ps = ppi.tile([128, CH], F32, tag="cells")
                for k in range(2):
                    jc = slice(k * 512, (k + 1) * 512)
                    nc.tensor.matmul(cells_ps[:, jc], emb_s[:, 0:128], oh_lo[:, jc],
                                     start=True, stop=False)
                    nc.tensor.matmul(cells_ps[:, jc], emb_s[:, 128:256], oh_hi[:, jc],
                                     start=False, stop=True)
                for kb in range(CH // 256):
                    # col j = t*256 + b_local  ->  t = j // 256
                    tt = (c0 + kb * 256) // 256
                    nc.scalar.activation(sig[:, c0 + kb * 256: c0 + (kb + 1) * 256],
                                         cells_ps[:, kb * 256:(kb + 1) * 256],
                                         AF.Identity, bias=posT_s[:, tt:tt + 1])

            # rule MLP: all tiny column ops (overlaps the gather pipeline)
            cp_s = mlp_sb.tile([128, 1], F32, tag="cp")
            nc.vector.tensor_reduce(cp_s[:], cT_s[:], axis=AX.X, op=ALU.add)
            y1_ps = ppi.tile([128, 2], F32, tag="mlp")
            for h in range(2):
                nc.tensor.matmul(y1_ps[:, h:h + 1], wc1_s[:, h * 128:(h + 1) * 128],
                                 cp_s[:], start=True, stop=True)
            y1g_s = mlp_sb.tile([128, 2], F32, tag="y1g")
            for h in range(2):
                nc.scalar.activation(y1g_s[:, h:h + 1], y1_ps[:, h:h + 1], AF.Gelu,
                                     bias=bc1_s[:, h:h + 1], scale=0.25)
            rb_ps = ppi.tile([128, 2], F32, tag="mlp")
            nc.tensor.matmul(rb_ps[:, 0:1], wc2_s[:, 0:128], y1g_s[:, 0:1],
                             start=True, stop=False)
            nc.tensor.matmul(rb_ps[:, 0:1], wc2_s[:, 128:256], y1g_s[:, 1:2],
                             start=False, stop=True)
            rb_s = mlp_sb.tile([128, 1], F32, tag="rb")
            nc.scalar.activation(rb_s[:], rb_ps[:, 0:1], AF.Identity, bias=bc2_s[:, 0:1])
            cb_ps = ppi.tile([128, 2], F32, tag="mlp")
            for h in range(2):
                nc.tensor.matmul(cb_ps[:, h:h + 1], w1b_s[:, h * 128:(h + 1) * 128],
                                 rb_s[:], start=True, stop=True)
            for h in range(2):
                nc.scalar.activation(cbias_s[:, h:h + 1], cb_ps[:, h:h + 1], AF.Identity,
                                     bias=b1_s[:, h:h + 1])

        # ---- evolve: 8 CA steps ----
        with tc.tile_pool(name="psum_pre", bufs=3, space="PSUM") as ppre, \
             tc.tile_pool(name="psum_new", bufs=1, space="PSUM") as pnew, \
             tc.tile_pool(name="h_sb", bufs=4) as sbh, \
             tc.tile_pool(name="t_sb", bufs=NCH + 1) as sbt:
            for s in range(NEV):
                # rotate chunk order so the next step's first chunks depend on
                # blends that completed early in this step
                order = [(s + j) % NCH for j in range(NCH)]
                t_tiles = {}
                for ci in order:
                    c0 = ci * CH
                    pre = [ppre.tile([128, CH], F32, tag="pre", name=f"pre{h_}")
                           for h_ in range(2)]
                    for h in range(2):
                        hcols = slice(h * 128, (h + 1) * 128)
                        for k in range(2):
                            d0 = c0 + k * 512
                            segs = [(wc_s, [(d0, d0, 512)]),
                                    (wl_s, _pieces(d0, 512, -256)),
                                    (wr_s, _pieces(d0, 512, +256))]
                            flat = [(w, dd, ss, ll) for w, ps in segs for dd, ss, ll in ps]
                            for i, (w, dd, ss, ll) in enumerate(flat):
                                nc.tensor.matmul(
                                    pre[h][:, dd - c0: dd - c0 + ll],
                                    w[:, hcols], sig[:, ss:ss + ll],
                                    start=(i == 0), stop=(i == len(flat) - 1))
                    h_t = [sbh.tile([128, CH], F32R, tag="h", name=f"ht{h_}")
                           for h_ in range(2)]
                    for h in range(2):
                        nc.scalar.activation(h_t[h][:], pre[h][:], AF.Gelu,
                                             bias=cbias_s[:, h:h + 1], scale=ia)
                    new_ps = pnew.tile([128, CH], F32, tag="new")
                    for k in range(2):
                        jc = slice(k * 512, (k + 1) * 512)
                        nc.tensor.matmul(new_ps[:, jc], w2_s[:, 0:128], h_t[0][:, jc],
                                         start=True, stop=False)
                        nc.tensor.matmul(new_ps[:, jc], w2_s[:, 128:256], h_t[1][:, jc],
                                         start=False, stop=True)
                    t_t = sbt.tile([128, CH], F32, tag="t")
                    nc.scalar.activation(t_t[:], new_ps[:], AF.Tanh, bias=b2_s[:, 0:1])
                    t_tiles[ci] = t_t
                # blends at end of step (keeps per-chunk pipeline free of
                # sigma-write hazards within the step)
                for ci in order:
                    c0 = ci * CH
                    nc.vector.scalar_tensor_tensor(
                        sig[:, c0:c0 + CH], sig[:, c0:c0 + CH], a, t_tiles[ci][:],
                        op0=ALU.mult, op1=ALU.add)

        # ---- final: LayerNorm stats + head ----
        with tc.tile_pool(name="psum_s1", bufs=1, space="PSUM") as ps1, \
             tc.tile_pool(name="psum_s2", bufs=1, space="PSUM") as ps2, \
             tc.tile_pool(name="psum_tm", bufs=1, space="PSUM") as ptm, \
             tc.tile_pool(name="psum_A", bufs=3, space="PSUM") as pA, \
             tc.tile_pool(name="sq_sb", bufs=2) as sbsq, \
             tc.tile_pool(name="srow_sb", bufs=3) as sbsr, \
             tc.tile_pool(name="stat_sb", bufs=1) as sbst, \
             tc.tile_pool(name="out_sb", bufs=4) as sbo:
            # prefetch the sqrt table set (Square/Copy live in it too) while
            # PE/DVE chew on stats; avoids a 2.7us ACT stall at the join.
            warm_s = sbst.tile([1, 8], F32, tag="warm")
            nc.scalar.activation(warm_s[:], fpack[0:1, 0:8], AF.Sqrt)

            stats_tm = ptm.tile([128, 2 * NBLK], F32, tag="stm")
            for ci in range(NCH):
                c0 = ci * CH
                sq_t = sbsq.tile([128, CH], F32R, tag="sq")
                nc.scalar.activation(sq_t[:], sig[:, c0:c0 + CH], AF.Square)
                sr1 = ps1.tile([1, CH], F32, tag="sr1")
                sr2 = ps2.tile([1, CH], F32, tag="sr2")
                for k in range(2):
                    jc = slice(k * 512, (k + 1) * 512)
                    nc.tensor.matmul(sr1[0:1, jc], ones_s[:, 0:1],
                                     sig[:, c0 + k * 512:c0 + (k + 1) * 512],
                                     start=True, stop=True)
                    nc.tensor.matmul(sr2[0:1, jc], ones_s[:, 0:1], sq_t[:, jc],
                                     start=True, stop=True)
                srow_t = sbsr.tile([1, 2 * CH], F32, tag="srow")
                nc.vector.tensor_copy(srow_t[0:1, 0:CH], sr1[0:1, :])
                nc.vector.tensor_copy(srow_t[0:1, CH:2 * CH], sr2[0:1, :])
                for j in range(CH // 128):
                    b = ci * (CH // 128) + j
                    nc.tensor.transpose(stats_tm[:, 2 * b:2 * b + 1],
                                        srow_t[0:1, j * 128:(j + 1) * 128],
                                        eye2_s[0:1, 0:1])
                    nc.tensor.transpose(stats_tm[:, 2 * b + 1:2 * b + 2],
                                        srow_t[0:1, CH + j * 128:CH + (j + 1) * 128],
                                        eye2_s[0:1, 0:1])
            # per-token inv-std on [128, 64] tiles (token-major)
            st3 = stats_tm[:].rearrange("p (b two) -> p b two", two=2)
            s1ap = st3[:, :, 0]
            s2ap = st3[:, :, 1]
            m2_s = sbst.tile([128, NBLK], F32, tag="m2")
            nc.scalar.activation(m2_s[:], s1ap, AF.Square, scale=ia / 128.0)
            vf_s = sbst.tile([128, NBLK], F32, tag="vf")
            nc.vector.scalar_tensor_tensor(vf_s[:], s2ap, ia * ia / 128.0, m2_s[:],
                                           op0=ALU.mult, op1=ALU.subtract)
            nc.vector.tensor_scalar_add(vf_s[:], vf_s[:], EPS)
            sd_s = sbst.tile([128, NBLK], F32, tag="sd")
            nc.scalar.activation(sd_s[:], vf_s[:], AF.Sqrt)
            y0_s = sbst.tile([128, NBLK], F32, tag="y0")
            nc.vector.reciprocal(y0_s[:], sd_s[:])
            q_s = sbst.tile([128, NBLK], F32, tag="q")
            nc.vector.tensor_mul(q_s[:], y0_s[:], y0_s[:])
            w_s = sbst.tile([128, NBLK], F32, tag="w")
            nc.vector.scalar_tensor_tensor(w_s[:], vf_s[:], -0.5, q_s[:],
                                           op0=ALU.mult, op1=ALU.mult)
            inv_s = sbst.tile([128, NBLK], F32, tag="inv")
            nc.vector.scalar_tensor_tensor(inv_s[:], w_s[:], 1.5, y0_s[:],
                                           op0=ALU.add, op1=ALU.mult)
            # head: out[tok, v] = inv[tok] * (sigma_blk^T @ hwc) (+ ln_b @ head_w)
            for b in range(NBLK):
                A_ps = pA.tile([128, V], F32, tag="A")
                nc.tensor.matmul(A_ps[:], sig[:, b * 128:(b + 1) * 128], hwc_s[:],
                                 start=True, stop=(not has_lnb))
                if has_lnb:
                    nc.tensor.matmul(A_ps[:], ones_s[0:1, 0:128], consth_r,
                                     start=False, stop=True)
                o_t = sbo.tile([128, V], F32, tag="o")
                nc.vector.tensor_scalar(o_t[:], A_ps[:], inv_s[:, b:b + 1], None,
                                        ALU.mult)
                tt = b // 2
                b0 = (b % 2) * 128
                nc.sync.dma_start(out_r[b0:b0 + 128, tt, :], o_t[:])

    nc.compile()
    return nc


def kernel(**inputs):
    g = {k: np.asarray(v, np.float32) if k != "tokens" else np.asarray(v)
         for k, v in inputs.items()}
    alpha = float(g["alpha"])
    a = float(1.0 / (1.0 + np.exp(-np.float64(alpha))))
    ia = 1.0 - a
    ln_b = g["ln_b"]
    has_lnb = bool(np.any(ln_b != 0))
    key = (np.float64(a).tobytes(), has_lnb)
    if key not in _CACHE:
        _CACHE[key] = _build(a, has_lnb)
    nc = _CACHE[key]

    W1, W2 = g["W1"], g["W2"]
    embed, pos = g["embed"], g["pos_embed"]
    head_w, ln_g = g["head_w"], g["ln_g"]

    wpack = np.zeros((128, WPACK_W), np.float32)
    wpack[:, 0:256] = W1[0:128]
    wpack[:, 256:512] = W1[128:256]
    wpack[:, 512:768] = W1[256:384]
    wpack[:, 768:1024] = np.concatenate([W2[0:128], W2[128:256]], axis=1)
    wpack[:, 1024:1280] = np.concatenate([embed[0:128], embed[128:256]],
                                         axis=1) * np.float32(1.0 / ia)
    ghw = head_w * ln_g[:, None]
    wpack[:, 1280:1536] = (ghw - ghw.mean(axis=0, keepdims=True)) * np.float32(ia)
    wpack[:, 1536:1664] = 1.0

    fpack = np.zeros((128, FPACK_W), np.float32)
    fpack[:, 0:256] = W1[384:512]
    fpack[:, 256:512] = g["Wc1"]
    fpack[:, 512:768] = np.concatenate([g["Wc2"][0:128], g["Wc2"][128:256]], axis=1)
    fpack[:, 768:800] = pos.T * np.float32(1.0 / ia)
    fpack[:, 800:804] = g["c_states"].T
    fpack[:, 804:806] = g["bc1"].reshape(2, 128).T
    fpack[:, 806:807] = g["bc2"].reshape(128, 1)
    fpack[:, 807:809] = g["b1"].reshape(2, 128).T
    fpack[:, 809:810] = g["b2"].reshape(128, 1)
    fpack[:, 810:812] = np.stack([np.arange(128), np.arange(128, 256)], axis=1)
    fpack[0:2, 812:814] = np.eye(2, dtype=np.float32)
    if has_lnb:
        fpack[0:1, 814:814 + 256] = (ln_b @ head_w).reshape(1, V)

    tokens = g["tokens"]
    in_maps = []
    for c in range(NC):
        tk = tokens[c * BL:(c + 1) * BL].astype(np.float32)   # (BL, T)
        in_maps.append({
            "tok": np.ascontiguousarray(tk.T).reshape(1, NTOK),  # t-major
            "wpack": wpack,
            "fpack": fpack,
        })

    kw = {}
    if TRACE:
        kw = dict(trace=True)
    res = run_bass_kernel_spmd(nc, in_maps, core_ids=list(range(NC)), **kw)
    if TRACE and res.exec_time_ns is not None:
        print(f"HW exec time: {res.exec_time_ns} ns")
        kernel.last_exec_ns = res.exec_time_ns
        kernel.last_trace = res.instructions_and_trace
    out = np.stack([res.results[c]["out"] for c in range(NC)], axis=0)
    return np.ascontiguousarray(out.reshape(B, T, V))
